# revision 10
# baseline (speedup 1.0000x reference)
"""Trainium2 Bass kernel for nn_Attention_16612933500996.

Full-input contract: kernel(**inputs) takes the unsharded inputs and returns
the full output. Internally shards across 8 NeuronCores: core i handles
batch b = i//2 and query-half w = i%2 (1024 of 2048 tokens). No collectives:
each core recomputes K/V for its whole batch (x rows are rotated host-side so
each core's query tokens are always rows 0..1023 — softmax over keys is
permutation invariant).

Per-core pipeline (all matmuls bf16 -> f32 PSUM):
  0. PE-transpose x [t,d] -> xT [d,t] (bf16)
  1. QKV projection: qT/kT produced transposed ([head*64+c, t]); V produced
     natural ([t, head-major cols]) with a fused ones-column per head so the
     attention U-matmul also yields the softmax denominator row.
  2. Attention per head: scoresT[m,w] = kT.T @ qT; exp via ACT (scores are
     ~±0.8 so no max-subtraction needed); U[65,w] = v_aug.T @ exp accumulated
     over key tiles (row 64 = sum of exps); normalize U/S with a PE-broadcast
     reciprocal; result nvT[e,w].
  3. Output projection (per-head K=64 accumulation) + bias + swish + residual
     + layernorm, DMA out.
"""

import sys

sys.path.insert(0, "/opt/trn_rl_repo")

import numpy as np

import concourse.bass as bass
import concourse.tile as tile
from concourse import mybir
from concourse.bass_utils import run_bass_kernel_spmd

AF = mybir.ActivationFunctionType
ALU = mybir.AluOpType
F32 = mybir.dt.float32
F32R = mybir.dt.float32r
BF16 = mybir.dt.bfloat16

B, L, D = 4, 2048, 1024
H, HD = 16, 64
WQ = 1024          # query tokens per core
N_CORES = 8
SCALE = 1.0 / float(np.sqrt(np.float32(L)))
LN_EPS = 1e-5


def _patch_tile_drain():
    """walrus in this container only accepts 1 sem wait on the TPB_CTRL drain;
    split the TileContext tail-drain waits across multiple drain instructions."""
    if getattr(tile.TileContext, "_drain_patched", False):
        return
    from concourse.tile import ScopedClock

    def _drain_and_barrier(self, tick_clock, wait_clock):
        nc = self.nc
        drain_inst = nc.sync.drain()
        wait_clock.add_sem_waits(
            drain_inst.ins, ScopedClock({None: tick_clock.global_clock})
        )
        si = drain_inst.ins.sync_info
        waits = list(si.on_wait) if si is not None else []
        MAXW = 1
        if len(waits) > MAXW:
            drain_inst.ins.sync_info = mybir.SyncInfo(
                on_wait=waits[:MAXW], on_update=list(si.on_update)
            )
            for i in range(MAXW, len(waits), MAXW):
                d2 = nc.sync.drain()
                d2.ins.sync_info = mybir.SyncInfo(
                    on_wait=waits[i : i + MAXW], on_update=[]
                )
        nc.all_engine_barrier()
        popped = nc._tile_sem_poison_stack.pop()
        assert popped is self._sem_poison
        nc.clear_and_free_semaphores(list(self.sems.allocated().values()))
        nc.all_engine_barrier()

    tile.TileContext._drain_and_barrier = _drain_and_barrier
    tile.TileContext._drain_patched = True


def _split_excess_waits(nc, max_waits=1):
    """walrus in this container has a tight per-instruction sync-wait slot
    limit; move excess waits onto same-engine nops preceding the instruction
    (same-engine queue order makes sequential waiting equivalent)."""
    for f in nc.m.functions:
        for bb in f.blocks:
            out = []
            changed = False
            for inst in bb.instructions:
                si = inst.sync_info
                waits = list(si.on_wait) if si is not None else []
                if len(waits) > max_waits:
                    lead = waits[: len(waits) - max_waits]
                    keep = waits[len(waits) - max_waits :]
                    for i in range(0, len(lead), max_waits):
                        nop = mybir.InstNoOp(
                            name=f"{inst.name}_w{i}", engine=inst.engine, ins=[], outs=[]
                        )
                        nop.sync_info = mybir.SyncInfo(
                            on_wait=lead[i : i + max_waits], on_update=[]
                        )
                        out.append(nop)
                    inst.sync_info = mybir.SyncInfo(
                        on_wait=keep, on_update=list(si.on_update)
                    )
                    changed = True
                out.append(inst)
            if changed:
                bb.instructions = out


def build_program(split_waits=True):
    _patch_tile_drain()
    nc = bass.Bass("TRN2", target_bir_lowering=False, debug=False, num_devices=N_CORES)

    xkv_d = nc.dram_tensor("xkv", [L, D], F32, kind="ExternalInput")
    wfc_d = nc.dram_tensor("wfc", [D, 3 * H * HD], F32, kind="ExternalInput")
    bfc_d = nc.dram_tensor("bfc", [3 * H * HD], F32, kind="ExternalInput")
    wfc2_d = nc.dram_tensor("wfc2", [H * HD, D], F32, kind="ExternalInput")
    bfc2_d = nc.dram_tensor("bfc2", [D], F32, kind="ExternalInput")
    ident_d = nc.dram_tensor("ident", [128, 128], F32, kind="ExternalInput")
    out_d = nc.dram_tensor("out", [WQ, D], F32, kind="ExternalOutput")

    NT = L // 128            # 16 token tiles
    ND = D // 128            # 8 d tiles
    NW = WQ // 128           # 8 query-token tiles
    NM = L // 128            # 16 key tiles

    with tile.TileContext(nc) as tc:
        pers = tc.alloc_tile_pool(name="pers", bufs=1)
        pmm = tc.alloc_tile_pool(name="pmm", bufs=2, space="PSUM")
        pu = tc.alloc_tile_pool(name="pu", bufs=2, space="PSUM")

        # --- constants ---
        ident = pers.tile([128, 128], BF16, tag="ident")
        nc.gpsimd.dma_start(ident[:, :], ident_d[:, :])
        ones = pers.tile([128, 128], BF16, tag="ones")
        nc.gpsimd.memset(ones[:, :], 1.0)
        eps = pers.tile([128, 1], F32, tag="eps")
        nc.gpsimd.memset(eps[:, :], LN_EPS)

        qkv_pool = tc.alloc_tile_pool(name="qkv", bufs=1)
        qT = [qkv_pool.tile([128, WQ], BF16, tag=f"qT{i}", name=f"qT{i}") for i in range(ND)]
        kT = [qkv_pool.tile([128, L], BF16, tag=f"kT{i}", name=f"kT{i}") for i in range(ND)]
        vaug = [qkv_pool.tile([128, H * 65], BF16, tag=f"va{i}", name=f"va{i}") for i in range(NM)]
        nvT = [pers.tile([64, WQ], BF16, tag=f"nv{h}", name=f"nv{h}") for h in range(H)]

        # ---------------- phase 0 + 1: transpose & QKV projection ----------
        with tc.tile_pool(name="ph1", bufs=1) as ph1:
            xkvT = [ph1.tile([128, L], BF16, tag=f"xkvT{i}", name=f"xkvT{i}") for i in range(ND)]
            for ti in range(NT):
                xb = ph1.tile([128, D], BF16, tag="xb", bufs=3)
                nc.gpsimd.dma_start(xb[:, :], xkv_d[ti * 128 : (ti + 1) * 128, :])
                for kd in range(ND):
                    pt = pmm.tile([128, 128], BF16, tag="mm")
                    nc.tensor.transpose(
                        pt[:, :], xb[:, kd * 128 : (kd + 1) * 128], ident[:, :]
                    )
                    nc.vector.tensor_copy(
                        xkvT[kd][:, ti * 128 : (ti + 1) * 128], pt[:, :]
                    )

            # q and k projections, transposed output [e-row, t]
            wfc_r = wfc_d.rearrange("d (h c) -> d h c", c=3 * HD)
            bfc_r = bfc_d.rearrange("(h c) -> h c", c=3 * HD)
            for et in range(2 * ND):
                is_q = et < ND
                qi = et % ND
                c0 = 0 if is_q else HD
                wts = []
                for kd in range(ND):
                    w = ph1.tile([128, 128], BF16, tag="wqk", bufs=10)
                    nc.gpsimd.dma_start(
                        w[:, :],
                        wfc_r[kd * 128 : (kd + 1) * 128, 2 * qi : 2 * qi + 2, c0 : c0 + HD],
                    )
                    wts.append(w)
                bt = ph1.tile([128, 1], F32, tag="bqk", bufs=3)
                nc.gpsimd.dma_start(bt[:, :], bfc_r[2 * qi : 2 * qi + 2, c0 : c0 + HD])
                ncols = WQ if is_q else L
                dst = qT[qi] if is_q else kT[qi]
                for half in range(ncols // 1024):
                    ps = pmm.tile([128, 1024], F32, tag="mm")
                    for tc2 in range(2):
                        t0 = half * 1024 + tc2 * 512
                        for kd in range(ND):
                            nc.tensor.matmul(
                                ps[:, tc2 * 512 : (tc2 + 1) * 512],
                                wts[kd][:, :],
                                xkvT[kd][:, t0 : t0 + 512],
                                start=(kd == 0),
                                stop=(kd == ND - 1),
                            )
                    nc.scalar.activation(
                        dst[:, half * 1024 : (half + 1) * 1024],
                        ps[:, :],
                        AF.Silu,
                        bias=bt[:, :],
                    )

            # v projection, natural layout with ones column fused per head
            wvs = {}
            for c2 in range(2):
                for kd in range(ND):
                    w = ph1.tile([128, 512], BF16, tag=f"wv{c2}_{kd}")
                    nc.gpsimd.dma_start(
                        w[:, :],
                        wfc_r[kd * 128 : (kd + 1) * 128, c2 * 8 : (c2 + 1) * 8, 2 * HD : 3 * HD],
                    )
                    wvs[(c2, kd)] = w
            bv = ph1.tile([1, H * HD], BF16, tag="bv")
            nc.gpsimd.dma_start(bv[:, :], bfc_r[:, 2 * HD : 3 * HD])
            for mt in range(NM):
                ps = pmm.tile([128, 1024], F32, tag="mm")
                for c2 in range(2):
                    sl = slice(c2 * 512, (c2 + 1) * 512)
                    for kd in range(ND):
                        nc.tensor.matmul(
                            ps[:, sl],
                            xkvT[kd][:, mt * 128 : (mt + 1) * 128],
                            wvs[(c2, kd)][:, :],
                            start=(kd == 0),
                            stop=False,
                        )
                    nc.tensor.matmul(
                        ps[:, sl],
                        ones[0:1, 0:128],
                        bv[0:1, sl],
                        start=False,
                        stop=True,
                    )
                va = vaug[mt]
                va_r = va[:, :].rearrange("p (h c) -> p h c", c=65)
                nc.gpsimd.memset(va_r[:, :, 64:65], 1.0)
                nc.scalar.activation(
                    va_r[:, :, 0:64],
                    ps[:, :],
                    AF.Silu,
                )

        # ---------------- phase 2: attention ------------------------------
        with tc.tile_pool(name="ph2", bufs=1) as ph2:
            for h in range(H):
                et = h // 2
                pr = (h % 2) * 64
                u = pu.tile([128, 1024], F32, tag="u")
                for mt in range(NM):
                    ps = pmm.tile([128, 1024], F32, tag="mm")
                    for wc in range(2):
                        nc.tensor.matmul(
                            ps[:, wc * 512 : (wc + 1) * 512],
                            kT[et][pr : pr + 64, mt * 128 : (mt + 1) * 128],
                            qT[et][pr : pr + 64, wc * 512 : (wc + 1) * 512],
                            start=True,
                            stop=True,
                        )
                    ex = ph2.tile([128, 1024], BF16, tag="exp", bufs=4)
                    nc.scalar.activation(ex[:, :], ps[:, :], AF.Exp, scale=SCALE)
                    for wc in range(2):
                        sl = slice(wc * 512, (wc + 1) * 512)
                        nc.tensor.matmul(
                            u[0:65, sl],
                            vaug[mt][:, h * 65 : (h + 1) * 65],
                            ex[:, sl],
                            start=(mt == 0),
                            stop=(mt == NM - 1),
                        )
                rc = ph2.tile([128, 1024], BF16, tag="recip", bufs=2)
                bcs = ph2.tile([64, 1024], F32, tag="bcs", bufs=2)
                bc = pmm.tile([128, 1024], F32, tag="mm")
                for wc in range(2):
                    sl = slice(wc * 512, (wc + 1) * 512)
                    with nc.allow_low_precision(reason="softmax denom recip bf16"):
                        nc.vector.reciprocal(rc[64:65, sl], u[64:65, sl])
                    nc.tensor.matmul(
                        bc[0:64, sl],
                        ones[64:65, 0:64],
                        rc[64:65, sl],
                        start=True,
                        stop=True,
                    )
                    nc.vector.tensor_copy(bcs[0:64, sl], bc[0:64, sl])
                    nc.vector.tensor_mul(
                        nvT[h][:, sl], u[0:64, sl], bcs[0:64, sl]
                    )

        # ---------------- phase 3: output projection + LN ------------------
        with tc.tile_pool(name="ph3", bufs=1) as ph3:
            w2 = []
            for h in range(H):
                w = ph3.tile([64, D], BF16, tag=f"w2_{h}")
                nc.gpsimd.dma_start(w[:, :], wfc2_d[h * HD : (h + 1) * HD, :])
                w2.append(w)
            b2 = ph3.tile([1, D], BF16, tag="b2")
            nc.gpsimd.dma_start(b2[:, :], bfc2_d[:])
            for wt in range(NW):
                po = pmm.tile([128, 1024], F32, tag="mm")
                for dc in range(2):
                    sl = slice(dc * 512, (dc + 1) * 512)
                    for h in range(H):
                        nc.tensor.matmul(
                            po[:, sl],
                            nvT[h][:, wt * 128 : (wt + 1) * 128],
                            w2[h][:, sl],
                            start=(h == 0),
                            stop=False,
                        )
                    nc.tensor.matmul(
                        po[:, sl],
                        ones[0:1, 0:128],
                        b2[0:1, sl],
                        start=False,
                        stop=True,
                    )
                msb = ph3.tile([128, 1024], F32, tag="m", bufs=2)
                nc.scalar.activation(msb[:, :], po[:, :], AF.Silu)
                xr = ph3.tile([128, 1024], F32, tag="xr", bufs=2)
                nc.gpsimd.dma_start(xr[:, :], xkv_d[wt * 128 : (wt + 1) * 128, :])
                y = ph3.tile([128, 1024], F32, tag="y", bufs=2)
                nc.vector.tensor_add(y[:, :], msb[:, :], xr[:, :])
                st = ph3.tile([128, 12], F32, tag="st", bufs=2)
                nc.vector.bn_stats(st[:, 0:6], y[:, 0:512])
                nc.vector.bn_stats(st[:, 6:12], y[:, 512:1024])
                mv = ph3.tile([128, 2], F32, tag="mv", bufs=2)
                nc.vector.bn_aggr(mv[:, :], st[:, :])
                sd = ph3.tile([128, 2], F32, tag="sd", bufs=2)
                nc.scalar.activation(sd[:, 0:1], mv[:, 1:2], AF.Sqrt, bias=eps[:, 0:1])
                nc.vector.reciprocal(sd[:, 1:2], sd[:, 0:1])
                ot = ph3.tile([128, 1024], F32, tag="ot", bufs=2)
                nc.vector.tensor_scalar(
                    ot[:, :],
                    y[:, :],
                    mv[:, 0:1],
                    sd[:, 1:2],
                    ALU.subtract,
                    ALU.mult,
                )
                nc.gpsimd.dma_start(out_d[wt * 128 : (wt + 1) * 128, :], ot[:, :])

        qkv_pool.release()
        pu.release()
        pmm.release()
        pers.release()

    if split_waits:
        _split_excess_waits(nc)
    return nc


_NC_CACHE = None


def _get_program():
    global _NC_CACHE
    if _NC_CACHE is None:
        _NC_CACHE = build_program()
    return _NC_CACHE


def make_in_maps(x, W_fc, b_fc, W_fc2, b_fc2):
    x = np.asarray(x, dtype=np.float32)
    W_fc = np.ascontiguousarray(np.asarray(W_fc, dtype=np.float32))
    b_fc = np.ascontiguousarray(np.asarray(b_fc, dtype=np.float32))
    W_fc2 = np.ascontiguousarray(np.asarray(W_fc2, dtype=np.float32))
    b_fc2 = np.ascontiguousarray(np.asarray(b_fc2, dtype=np.float32))
    ident = np.eye(128, dtype=np.float32)
    in_maps = []
    for i in range(N_CORES):
        b = i // 2
        w0 = (i % 2) * WQ
        xkv = np.ascontiguousarray(
            np.concatenate([x[b, w0:], x[b, :w0]], axis=0)
        )
        in_maps.append(
            {
                "xkv": xkv,
                "wfc": W_fc,
                "bfc": b_fc,
                "wfc2": W_fc2,
                "bfc2": b_fc2,
                "ident": ident,
            }
        )
    return in_maps


def kernel(x, W_fc, b_fc, W_fc2, b_fc2, **extra):
    nc = _get_program()
    in_maps = make_in_maps(x, W_fc, b_fc, W_fc2, b_fc2)
    res = run_bass_kernel_spmd(nc, in_maps, list(range(N_CORES)))
    out = np.empty((B, L, D), dtype=np.float32)
    for i in range(N_CORES):
        b = i // 2
        w0 = (i % 2) * WQ
        out[b, w0 : w0 + WQ] = res.results[i]["out"]
    return out


# revision 11
# speedup vs baseline: 1.1718x; 1.1718x over previous
"""Trainium2 Bass kernel for nn_Attention_16612933500996.

Full-input contract: kernel(**inputs) takes the unsharded inputs and returns
the full output. Internally shards across 8 NeuronCores: core i handles
batch b = i//2 and query-half w = i%2 (1024 of 2048 tokens). No collectives:
each core recomputes K/V for its whole batch (x rows are rotated host-side so
each core's query tokens are always rows 0..1023 — softmax over keys is
permutation invariant).

Per-core pipeline (all matmuls bf16 -> f32 PSUM):
  0. PE-transpose x [t,d] -> xT [d,t] (bf16)
  1. QKV projection: qT/kT produced transposed ([head*64+c, t]); V produced
     natural ([t, head-major cols]) with a fused ones-column per head so the
     attention U-matmul also yields the softmax denominator row.
  2. Attention per head: scoresT[m,w] = kT.T @ qT; exp via ACT (scores are
     ~±0.8 so no max-subtraction needed); U[65,w] = v_aug.T @ exp accumulated
     over key tiles (row 64 = sum of exps); normalize U/S with a PE-broadcast
     reciprocal; result nvT[e,w].
  3. Output projection (per-head K=64 accumulation) + bias + swish + residual
     + layernorm, DMA out.
"""

import sys

sys.path.insert(0, "/opt/trn_rl_repo")

import numpy as np

import concourse.bass as bass
import concourse.tile as tile
from concourse import mybir
from concourse.bass_utils import run_bass_kernel_spmd

AF = mybir.ActivationFunctionType
ALU = mybir.AluOpType
F32 = mybir.dt.float32
F32R = mybir.dt.float32r
BF16 = mybir.dt.bfloat16

B, L, D = 4, 2048, 1024
H, HD = 16, 64
WQ = 1024          # query tokens per core
N_CORES = 8
SCALE = 1.0 / float(np.sqrt(np.float32(L)))
LN_EPS = 1e-5


def _patch_tile_drain():
    """walrus in this container only accepts 1 sem wait on the TPB_CTRL drain;
    split the TileContext tail-drain waits across multiple drain instructions."""
    if getattr(tile.TileContext, "_drain_patched", False):
        return
    from concourse.tile import ScopedClock

    def _drain_and_barrier(self, tick_clock, wait_clock):
        nc = self.nc
        drain_inst = nc.sync.drain()
        wait_clock.add_sem_waits(
            drain_inst.ins, ScopedClock({None: tick_clock.global_clock})
        )
        si = drain_inst.ins.sync_info
        waits = list(si.on_wait) if si is not None else []
        MAXW = 1
        if len(waits) > MAXW:
            drain_inst.ins.sync_info = mybir.SyncInfo(
                on_wait=waits[:MAXW], on_update=list(si.on_update)
            )
            for i in range(MAXW, len(waits), MAXW):
                d2 = nc.sync.drain()
                d2.ins.sync_info = mybir.SyncInfo(
                    on_wait=waits[i : i + MAXW], on_update=[]
                )
        nc.all_engine_barrier()
        popped = nc._tile_sem_poison_stack.pop()
        assert popped is self._sem_poison
        nc.clear_and_free_semaphores(list(self.sems.allocated().values()))
        nc.all_engine_barrier()

    tile.TileContext._drain_and_barrier = _drain_and_barrier
    tile.TileContext._drain_patched = True


def _split_excess_waits(nc, max_waits=1):
    """walrus in this container has a tight per-instruction sync-wait slot
    limit; move excess waits onto same-engine nops preceding the instruction
    (same-engine queue order makes sequential waiting equivalent)."""
    for f in nc.m.functions:
        for bb in f.blocks:
            out = []
            changed = False
            for inst in bb.instructions:
                si = inst.sync_info
                waits = list(si.on_wait) if si is not None else []
                if len(waits) > max_waits:
                    lead = waits[: len(waits) - max_waits]
                    keep = waits[len(waits) - max_waits :]
                    for i in range(0, len(lead), max_waits):
                        nop = mybir.InstNoOp(
                            name=f"{inst.name}_w{i}", engine=inst.engine, ins=[], outs=[]
                        )
                        nop.sync_info = mybir.SyncInfo(
                            on_wait=lead[i : i + max_waits], on_update=[]
                        )
                        out.append(nop)
                    inst.sync_info = mybir.SyncInfo(
                        on_wait=keep, on_update=list(si.on_update)
                    )
                    changed = True
                out.append(inst)
            if changed:
                bb.instructions = out


def build_program(split_waits=True):
    _patch_tile_drain()
    nc = bass.Bass("TRN2", target_bir_lowering=False, debug=False, num_devices=N_CORES)

    xkv_d = nc.dram_tensor("xkv", [L, D], F32, kind="ExternalInput")
    wfc_d = nc.dram_tensor("wfc", [D, 3 * H * HD], F32, kind="ExternalInput")
    bfc_d = nc.dram_tensor("bfc", [3 * H * HD], F32, kind="ExternalInput")
    wfc2_d = nc.dram_tensor("wfc2", [H * HD, D], F32, kind="ExternalInput")
    bfc2_d = nc.dram_tensor("bfc2", [D], F32, kind="ExternalInput")
    ident_d = nc.dram_tensor("ident", [128, 128], F32, kind="ExternalInput")
    out_d = nc.dram_tensor("out", [WQ, D], F32, kind="ExternalOutput")

    NT = L // 128            # 16 token tiles
    ND = D // 128            # 8 d tiles
    NW = WQ // 128           # 8 query-token tiles
    NM = L // 128            # 16 key tiles

    with tile.TileContext(nc) as tc:
        pers = tc.alloc_tile_pool(name="pers", bufs=1)
        pmm = tc.alloc_tile_pool(name="pmm", bufs=3, space="PSUM")
        pu = tc.alloc_tile_pool(name="pu", bufs=1, space="PSUM")

        # --- constants ---
        ident = pers.tile([128, 128], BF16, tag="ident")
        nc.gpsimd.dma_start(ident[:, :], ident_d[:, :])
        ones = pers.tile([128, 128], BF16, tag="ones")
        nc.gpsimd.memset(ones[:, :], 1.0)
        eps = pers.tile([128, 1], F32, tag="eps")
        nc.gpsimd.memset(eps[:, :], LN_EPS)

        qkv_pool = tc.alloc_tile_pool(name="qkv", bufs=1)
        qT = [qkv_pool.tile([128, WQ], BF16, tag=f"qT{i}", name=f"qT{i}") for i in range(ND)]
        kT = [qkv_pool.tile([128, L], BF16, tag=f"kT{i}", name=f"kT{i}") for i in range(ND)]
        vaug = [qkv_pool.tile([128, H * 65], BF16, tag=f"va{i}", name=f"va{i}") for i in range(NM)]
        nvT = [pers.tile([64, WQ], BF16, tag=f"nv{h}", name=f"nv{h}") for h in range(H)]

        # ---- phases 0-2 interleaved: transpose, v-proj, then per head-pair
        # q/k projection immediately followed by that pair's attention, so the
        # PE keeps dense work while ACT chews through the exps.
        with tc.tile_pool(name="ph12", bufs=1) as ph1:
            ph2 = ph1
            xkvT = [ph1.tile([128, L], BF16, tag=f"xkvT{i}", name=f"xkvT{i}") for i in range(ND)]
            for ti in range(NT):
                xb = ph1.tile([128, D], BF16, tag="xb", bufs=3)
                nc.gpsimd.dma_start(xb[:, :], xkv_d[ti * 128 : (ti + 1) * 128, :])
                for kd in range(ND):
                    pt = pmm.tile([128, 128], BF16, tag="mm")
                    nc.tensor.transpose(
                        pt[:, :], xb[:, kd * 128 : (kd + 1) * 128], ident[:, :]
                    )
                    nc.vector.tensor_copy(
                        xkvT[kd][:, ti * 128 : (ti + 1) * 128], pt[:, :]
                    )

            wfc_r = wfc_d.rearrange("d (h c) -> d h c", c=3 * HD)
            bfc_r = bfc_d.rearrange("(h c) -> h c", c=3 * HD)

            # v projection first (attention needs all of v_aug)
            wvs = {}
            for c2 in range(2):
                for kd in range(ND):
                    w = ph1.tile([128, 512], BF16, tag=f"wv{c2}_{kd}")
                    nc.gpsimd.dma_start(
                        w[:, :],
                        wfc_r[kd * 128 : (kd + 1) * 128, c2 * 8 : (c2 + 1) * 8, 2 * HD : 3 * HD],
                    )
                    wvs[(c2, kd)] = w
            bv = ph1.tile([1, H * HD], BF16, tag="bv")
            nc.gpsimd.dma_start(bv[:, :], bfc_r[:, 2 * HD : 3 * HD])
            for mt in range(NM):
                ps = pmm.tile([128, 1024], F32, tag="mm")
                for c2 in range(2):
                    sl = slice(c2 * 512, (c2 + 1) * 512)
                    for kd in range(ND):
                        nc.tensor.matmul(
                            ps[:, sl],
                            xkvT[kd][:, mt * 128 : (mt + 1) * 128],
                            wvs[(c2, kd)][:, :],
                            start=(kd == 0),
                            stop=False,
                        )
                    nc.tensor.matmul(
                        ps[:, sl],
                        ones[0:1, 0:128],
                        bv[0:1, sl],
                        start=False,
                        stop=True,
                    )
                va = vaug[mt]
                va_r = va[:, :].rearrange("p (h c) -> p h c", c=65)
                nc.gpsimd.memset(va_r[:, :, 64:65], 1.0)
                nc.scalar.activation(
                    va_r[:, :, 0:64],
                    ps[:, :],
                    AF.Silu,
                )

            def project_qk(et):
                """q (et<ND) or k (et>=ND) projection for e-tile et%ND."""
                is_q = et < ND
                qi = et % ND
                c0 = 0 if is_q else HD
                wts = []
                for kd in range(ND):
                    w = ph1.tile([128, 128], BF16, tag="wqk", bufs=10, name=f"wqk{et}_{kd}")
                    nc.gpsimd.dma_start(
                        w[:, :],
                        wfc_r[kd * 128 : (kd + 1) * 128, 2 * qi : 2 * qi + 2, c0 : c0 + HD],
                    )
                    wts.append(w)
                bt = ph1.tile([128, 1], F32, tag="bqk", bufs=3, name=f"bqk{et}")
                nc.gpsimd.dma_start(bt[:, :], bfc_r[2 * qi : 2 * qi + 2, c0 : c0 + HD])
                ncols = WQ if is_q else L
                dst = qT[qi] if is_q else kT[qi]
                for half in range(ncols // 1024):
                    ps = pmm.tile([128, 1024], F32, tag="mm", name=f"qk{et}_{half}")
                    for tc2 in range(2):
                        t0 = half * 1024 + tc2 * 512
                        for kd in range(ND):
                            nc.tensor.matmul(
                                ps[:, tc2 * 512 : (tc2 + 1) * 512],
                                wts[kd][:, :],
                                xkvT[kd][:, t0 : t0 + 512],
                                start=(kd == 0),
                                stop=(kd == ND - 1),
                            )
                    nc.scalar.activation(
                        dst[:, half * 1024 : (half + 1) * 1024],
                        ps[:, :],
                        AF.Silu,
                        bias=bt[:, :],
                    )

            def attention(h):
                et = h // 2
                pr = (h % 2) * 64
                u = pu.tile([128, 1024], F32, tag="u", name=f"u{h}")
                for mt in range(NM):
                    ps = pmm.tile([128, 1024], F32, tag="mm", name=f"sc{h}_{mt}")
                    for wc in range(2):
                        nc.tensor.matmul(
                            ps[:, wc * 512 : (wc + 1) * 512],
                            kT[et][pr : pr + 64, mt * 128 : (mt + 1) * 128],
                            qT[et][pr : pr + 64, wc * 512 : (wc + 1) * 512],
                            start=True,
                            stop=True,
                        )
                    ex = ph2.tile([128, 1024], BF16, tag="exp", bufs=4, name=f"ex{h}_{mt}")
                    nc.scalar.activation(ex[:, :], ps[:, :], AF.Exp, scale=SCALE)
                    for wc in range(2):
                        sl = slice(wc * 512, (wc + 1) * 512)
                        nc.tensor.matmul(
                            u[0:65, sl],
                            vaug[mt][:, h * 65 : (h + 1) * 65],
                            ex[:, sl],
                            start=(mt == 0),
                            stop=(mt == NM - 1),
                        )
                rc = ph2.tile([128, 1024], BF16, tag="recip", bufs=2, name=f"rc{h}")
                bcs = ph2.tile([64, 1024], F32, tag="bcs", bufs=2, name=f"bcs{h}")
                bc = pmm.tile([128, 1024], F32, tag="mm", name=f"bc{h}")
                for wc in range(2):
                    sl = slice(wc * 512, (wc + 1) * 512)
                    with nc.allow_low_precision(reason="softmax denom recip bf16"):
                        nc.vector.reciprocal(rc[64:65, sl], u[64:65, sl])
                    nc.tensor.matmul(
                        bc[0:64, sl],
                        ones[64:65, 0:64],
                        rc[64:65, sl],
                        start=True,
                        stop=True,
                    )
                    nc.vector.tensor_copy(bcs[0:64, sl], bc[0:64, sl])
                    nc.vector.tensor_mul(
                        nvT[h][:, sl], u[0:64, sl], bcs[0:64, sl]
                    )

            for et in range(ND):
                project_qk(et)        # q for head pair (2et, 2et+1)
                project_qk(ND + et)   # k for head pair
                attention(2 * et)
                attention(2 * et + 1)

        # ---------------- phase 3: output projection + LN ------------------
        with tc.tile_pool(name="ph3", bufs=1) as ph3:
            w2 = []
            for h in range(H):
                w = ph3.tile([64, D], BF16, tag=f"w2_{h}")
                nc.gpsimd.dma_start(w[:, :], wfc2_d[h * HD : (h + 1) * HD, :])
                w2.append(w)
            b2 = ph3.tile([1, D], BF16, tag="b2")
            nc.gpsimd.dma_start(b2[:, :], bfc2_d[:])
            for wt in range(NW):
                po = pmm.tile([128, 1024], F32, tag="mm")
                for dc in range(2):
                    sl = slice(dc * 512, (dc + 1) * 512)
                    for h in range(H):
                        nc.tensor.matmul(
                            po[:, sl],
                            nvT[h][:, wt * 128 : (wt + 1) * 128],
                            w2[h][:, sl],
                            start=(h == 0),
                            stop=False,
                        )
                    nc.tensor.matmul(
                        po[:, sl],
                        ones[0:1, 0:128],
                        b2[0:1, sl],
                        start=False,
                        stop=True,
                    )
                msb = ph3.tile([128, 1024], F32, tag="m", bufs=2)
                nc.scalar.activation(msb[:, :], po[:, :], AF.Silu)
                xr = ph3.tile([128, 1024], F32, tag="xr", bufs=2)
                nc.gpsimd.dma_start(xr[:, :], xkv_d[wt * 128 : (wt + 1) * 128, :])
                y = ph3.tile([128, 1024], F32, tag="y", bufs=2)
                nc.vector.tensor_add(y[:, :], msb[:, :], xr[:, :])
                st = ph3.tile([128, 12], F32, tag="st", bufs=2)
                nc.vector.bn_stats(st[:, 0:6], y[:, 0:512])
                nc.vector.bn_stats(st[:, 6:12], y[:, 512:1024])
                mv = ph3.tile([128, 2], F32, tag="mv", bufs=2)
                nc.vector.bn_aggr(mv[:, :], st[:, :])
                sd = ph3.tile([128, 2], F32, tag="sd", bufs=2)
                nc.scalar.activation(sd[:, 0:1], mv[:, 1:2], AF.Sqrt, bias=eps[:, 0:1])
                nc.vector.reciprocal(sd[:, 1:2], sd[:, 0:1])
                ot = ph3.tile([128, 1024], F32, tag="ot", bufs=2)
                nc.vector.tensor_scalar(
                    ot[:, :],
                    y[:, :],
                    mv[:, 0:1],
                    sd[:, 1:2],
                    ALU.subtract,
                    ALU.mult,
                )
                nc.gpsimd.dma_start(out_d[wt * 128 : (wt + 1) * 128, :], ot[:, :])

        qkv_pool.release()
        pu.release()
        pmm.release()
        pers.release()

    if split_waits:
        _split_excess_waits(nc)
    return nc


_NC_CACHE = None


def _get_program():
    global _NC_CACHE
    if _NC_CACHE is None:
        _NC_CACHE = build_program()
    return _NC_CACHE


def make_in_maps(x, W_fc, b_fc, W_fc2, b_fc2):
    x = np.asarray(x, dtype=np.float32)
    W_fc = np.ascontiguousarray(np.asarray(W_fc, dtype=np.float32))
    b_fc = np.ascontiguousarray(np.asarray(b_fc, dtype=np.float32))
    W_fc2 = np.ascontiguousarray(np.asarray(W_fc2, dtype=np.float32))
    b_fc2 = np.ascontiguousarray(np.asarray(b_fc2, dtype=np.float32))
    ident = np.eye(128, dtype=np.float32)
    in_maps = []
    for i in range(N_CORES):
        b = i // 2
        w0 = (i % 2) * WQ
        xkv = np.ascontiguousarray(
            np.concatenate([x[b, w0:], x[b, :w0]], axis=0)
        )
        in_maps.append(
            {
                "xkv": xkv,
                "wfc": W_fc,
                "bfc": b_fc,
                "wfc2": W_fc2,
                "bfc2": b_fc2,
                "ident": ident,
            }
        )
    return in_maps


def kernel(x, W_fc, b_fc, W_fc2, b_fc2, **extra):
    nc = _get_program()
    in_maps = make_in_maps(x, W_fc, b_fc, W_fc2, b_fc2)
    res = run_bass_kernel_spmd(nc, in_maps, list(range(N_CORES)))
    out = np.empty((B, L, D), dtype=np.float32)
    for i in range(N_CORES):
        b = i // 2
        w0 = (i % 2) * WQ
        out[b, w0 : w0 + WQ] = res.results[i]["out"]
    return out
